# revision 5
# baseline (speedup 1.0000x reference)
"""Discrete VAE (VQ codebook) kernel for 8 Trainium2 NeuronCores.

Pipeline per core (data-parallel over batch, 1024 tokens/core):
  scores = xT.T @ cbT + (-0.5*||c||^2)      # argmin d2 == argmax scores
  ids    = argmax(scores)                    # vector.max + max_index
  q      = codebook[ids]                     # indirect DMA gather
  h1T    = relu(w1 @ q.T + b1); h2T = relu(w2 @ h1 + b2); recT = w3 @ h2 + b3
  chamfer(rec, gt) accumulated per token, summed on host.
"""

import sys

if "/opt/trn_rl_repo" not in sys.path:
    sys.path.insert(0, "/opt/trn_rl_repo")

import os
import numpy as np

from concourse import bacc, bass, mybir
from concourse.bass import IndirectOffsetOnAxis
from concourse.masks import make_identity
from concourse.tile import TileContext
from concourse.bass_utils import run_bass_kernel_spmd

B, G, K, C, NT = 128, 64, 32, 256, 8192
NCORES = 8
TOK_PER_CORE = B * G // NCORES  # 1024
NTILES = TOK_PER_CORE // 128  # 8
NCHUNK = NT // 512  # 16

F32 = mybir.dt.float32
F32R = mybir.dt.float32r
U32 = mybir.dt.uint32

# set to False to fall back to plain fp32 matmuls for the distance scores
SCORES_F32R = os.environ.get("KERNEL_SCORES_F32R", "1") == "1"

_CACHE = {}


def _build():
    if "nc" in _CACHE:
        return _CACHE["nc"]

    nc = bacc.Bacc("TRN2", target_bir_lowering=False, debug=False,
                   num_devices=NCORES)

    MMDT = F32R if SCORES_F32R else F32
    xT = nc.dram_tensor("xT", [C, TOK_PER_CORE], MMDT, kind="ExternalInput")
    cbT = nc.dram_tensor("cbT", [C, NT], MMDT, kind="ExternalInput")
    cb = nc.dram_tensor("cb", [NT, C], F32, kind="ExternalInput")
    cnorm = nc.dram_tensor("cnorm", [1, NT], F32, kind="ExternalInput")
    w1T = nc.dram_tensor("w1T", [C, 512], F32, kind="ExternalInput")
    w2T = nc.dram_tensor("w2T", [512, C], F32, kind="ExternalInput")
    w3T = nc.dram_tensor("w3T", [C, 3 * K], F32, kind="ExternalInput")
    b1 = nc.dram_tensor("b1", [512, 1], F32, kind="ExternalInput")
    b2 = nc.dram_tensor("b2", [C, 1], F32, kind="ExternalInput")
    b3 = nc.dram_tensor("b3", [3 * K, 1], F32, kind="ExternalInput")
    gt = nc.dram_tensor("gt", [TOK_PER_CORE, 3 * K], F32, kind="ExternalInput")
    out = nc.dram_tensor("out", [128, K], F32, kind="ExternalOutput")

    with TileContext(nc) as tc:
        with (
            tc.tile_pool(name="const", bufs=1) as cpool,
            tc.tile_pool(name="scores", bufs=1) as spool,
            tc.tile_pool(name="work", bufs=2) as wpool,
            tc.tile_pool(name="mlp", bufs=8) as mpool,
            tc.tile_pool(name="cham", bufs=1) as chpool,
            tc.tile_pool(name="ps_score", bufs=2, space="PSUM") as ps_s,
            tc.tile_pool(name="ps_small", bufs=4, space="PSUM") as ps_m,
        ):
            # ---- resident constants ----
            ident = cpool.tile([128, 128], F32, tag="ident")
            make_identity(nc, ident[:])

            cbT_sb = []
            for kk in range(2):
                t = cpool.tile([128, NT], MMDT, tag=f"cbT{kk}")
                nc.gpsimd.dma_start(out=t[:], in_=cbT[kk * 128:(kk + 1) * 128, :])
                cbT_sb.append(t)

            cnorm_bc = cpool.tile([128, NT], F32, tag="cnorm")
            nc.gpsimd.dma_start(
                out=cnorm_bc[:], in_=cnorm[0:1, :].to_broadcast([128, NT])
            )

            w1_sb = []
            for kk in range(2):
                t = cpool.tile([128, 512], F32, tag=f"w1_{kk}")
                nc.gpsimd.dma_start(out=t[:], in_=w1T[kk * 128:(kk + 1) * 128, :])
                w1_sb.append(t)
            w2_sb = []
            for kk in range(4):
                t = cpool.tile([128, C], F32, tag=f"w2_{kk}")
                nc.gpsimd.dma_start(out=t[:], in_=w2T[kk * 128:(kk + 1) * 128, :])
                w2_sb.append(t)
            w3_sb = []
            for kk in range(2):
                t = cpool.tile([128, 3 * K], F32, tag=f"w3_{kk}")
                nc.gpsimd.dma_start(out=t[:], in_=w3T[kk * 128:(kk + 1) * 128, :])
                w3_sb.append(t)
            b1_sb = []
            for m in range(4):
                t = cpool.tile([128, 1], F32, tag=f"b1_{m}")
                nc.gpsimd.dma_start(out=t[:], in_=b1[m * 128:(m + 1) * 128, :])
                b1_sb.append(t)
            b2_sb = []
            for m in range(2):
                t = cpool.tile([128, 1], F32, tag=f"b2_{m}")
                nc.gpsimd.dma_start(out=t[:], in_=b2[m * 128:(m + 1) * 128, :])
                b2_sb.append(t)
            b3_sb = cpool.tile([3 * K, 1], F32, tag="b3")
            nc.gpsimd.dma_start(out=b3_sb[:], in_=b3[:, :])

            acc = cpool.tile([128, K], F32, tag="acc")
            nc.vector.memset(acc[:], 0.0)

            # ---- per token-tile pipeline ----
            for t in range(NTILES):
                ts = slice(t * 128, (t + 1) * 128)

                xt = []
                for kk in range(2):
                    x = wpool.tile([128, 128], MMDT, tag="xt")
                    nc.gpsimd.dma_start(out=x[:], in_=xT[kk * 128:(kk + 1) * 128, ts])
                    xt.append(x)

                scores = spool.tile([128, NT], F32, tag="scores")
                for ch in range(NCHUNK):
                    cs = slice(ch * 512, (ch + 1) * 512)
                    ps = ps_s.tile([128, 512], F32, tag="ps_score")
                    nc.tensor.matmul(ps[:], lhsT=xt[0][:],
                                     rhs=cbT_sb[0][:, cs],
                                     start=True, stop=False)
                    nc.tensor.matmul(ps[:], lhsT=xt[1][:],
                                     rhs=cbT_sb[1][:, cs],
                                     start=False, stop=True)
                    nc.vector.tensor_tensor(out=scores[:, cs], in0=ps[:],
                                            in1=cnorm_bc[:, cs],
                                            op=mybir.AluOpType.add)

                max8 = wpool.tile([128, 8], F32, tag="max8")
                nc.vector.max(out=max8[:], in_=scores[:])
                idx8 = wpool.tile([128, 8], U32, tag="idx8")
                nc.vector.max_index(out=idx8[:], in_max=max8[:], in_values=scores[:])

                q = wpool.tile([128, C], F32, tag="q")
                nc.gpsimd.indirect_dma_start(
                    out=q[:], out_offset=None, in_=cb[:, :],
                    in_offset=IndirectOffsetOnAxis(ap=idx8[:, 0:1], axis=0),
                )

                qT = []
                for kk in range(2):
                    pt = ps_m.tile([128, 128], F32, tag="ps_small")
                    nc.tensor.transpose(out=pt[:], in_=q[:, kk * 128:(kk + 1) * 128],
                                        identity=ident[:])
                    qt = mpool.tile([128, 128], F32, tag="qT")
                    nc.scalar.activation(out=qt[:], in_=pt[:],
                                         func=mybir.ActivationFunctionType.Copy)
                    qT.append(qt)

                h1 = []
                for m in range(4):
                    ph = ps_m.tile([128, 128], F32, tag="ps_small")
                    for kk in range(2):
                        nc.tensor.matmul(ph[:],
                                         lhsT=w1_sb[kk][:, m * 128:(m + 1) * 128],
                                         rhs=qT[kk][:],
                                         start=(kk == 0), stop=(kk == 1))
                    ht = mpool.tile([128, 128], F32, tag="h1")
                    nc.scalar.activation(out=ht[:], in_=ph[:],
                                         func=mybir.ActivationFunctionType.Relu,
                                         bias=b1_sb[m][:])
                    h1.append(ht)

                h2 = []
                for m in range(2):
                    ph = ps_m.tile([128, 128], F32, tag="ps_small")
                    for kk in range(4):
                        nc.tensor.matmul(ph[:],
                                         lhsT=w2_sb[kk][:, m * 128:(m + 1) * 128],
                                         rhs=h1[kk][:],
                                         start=(kk == 0), stop=(kk == 3))
                    ht = mpool.tile([128, 128], F32, tag="h2")
                    nc.scalar.activation(out=ht[:], in_=ph[:],
                                         func=mybir.ActivationFunctionType.Relu,
                                         bias=b2_sb[m][:])
                    h2.append(ht)

                pr = ps_m.tile([96, 128], F32, tag="ps_small")
                for kk in range(2):
                    nc.tensor.matmul(pr[:], lhsT=w3_sb[kk][:], rhs=h2[kk][:],
                                     start=(kk == 0), stop=(kk == 1))
                recT = mpool.tile([96, 128], F32, tag="recT")
                nc.scalar.activation(out=recT[:], in_=pr[:],
                                     func=mybir.ActivationFunctionType.Identity,
                                     bias=b3_sb[:])

                prt = ps_m.tile([128, 96], F32, tag="ps_small")
                nc.tensor.transpose(out=prt[:], in_=recT[:],
                                    identity=ident[0:96, 0:96])
                rec = wpool.tile([128, 96], F32, tag="rec")
                nc.vector.tensor_copy(out=rec[:], in_=prt[:])

                gtt = wpool.tile([128, 96], F32, tag="gt")
                nc.gpsimd.dma_start(out=gtt[:], in_=gt[ts, :])

                # chamfer: dif[t, i, j, c] = rec[t,i,c] - gt[t,j,c]
                dif = chpool.tile([128, K * K * 3], F32, tag="dif")
                rec_b = (rec[:].rearrange("p (i c) -> p i c", c=3)
                         .unsqueeze(2).broadcast_to([128, K, K, 3]))
                gt_b = (gtt[:].rearrange("p (j c) -> p j c", c=3)
                        .unsqueeze(1).broadcast_to([128, K, K, 3]))
                dif4 = dif[:].rearrange("p (i j c) -> p i j c", j=K, c=3)
                nc.vector.tensor_tensor(out=dif4, in0=rec_b, in1=gt_b,
                                        op=mybir.AluOpType.subtract)
                nc.scalar.activation(out=dif[:], in_=dif[:],
                                     func=mybir.ActivationFunctionType.Square)

                dd = chpool.tile([128, K * K], F32, tag="dd")
                nc.vector.tensor_reduce(
                    out=dd[:], in_=dif[:].rearrange("p (ij c) -> p ij c", c=3),
                    axis=mybir.AxisListType.X, op=mybir.AluOpType.add)

                dd3 = dd[:].rearrange("p (i j) -> p i j", j=K)
                minj = chpool.tile([128, K], F32, tag="minj")
                nc.vector.tensor_reduce(out=minj[:], in_=dd3,
                                        axis=mybir.AxisListType.X,
                                        op=mybir.AluOpType.min)
                mini = chpool.tile([128, K], F32, tag="mini")
                nc.vector.tensor_reduce(out=mini[:], in_=dd3.transpose([0, 2, 1]),
                                        axis=mybir.AxisListType.X,
                                        op=mybir.AluOpType.min)
                nc.vector.tensor_tensor(out=acc[:], in0=acc[:], in1=minj[:],
                                        op=mybir.AluOpType.add)
                nc.vector.tensor_tensor(out=acc[:], in0=acc[:], in1=mini[:],
                                        op=mybir.AluOpType.add)

            nc.gpsimd.dma_start(out=out[:, :], in_=acc[:])

    nc.compile()
    _CACHE["nc"] = nc
    return nc


def kernel(patch_features, neighborhood, codebook, w1, b1, w2, b2, w3, b3):
    nc = _build()

    x = np.ascontiguousarray(
        np.asarray(patch_features, np.float32).reshape(B * G, C))
    gt_full = np.ascontiguousarray(
        np.asarray(neighborhood, np.float32).reshape(B * G, 3 * K))
    cbk = np.ascontiguousarray(np.asarray(codebook, np.float32))
    cbT_h = np.ascontiguousarray(cbk.T)
    cnorm_h = np.ascontiguousarray(
        (-0.5 * (cbk.astype(np.float64) ** 2).sum(1)).astype(np.float32)
        .reshape(1, NT))
    w1T_h = np.ascontiguousarray(np.asarray(w1, np.float32).T)
    w2T_h = np.ascontiguousarray(np.asarray(w2, np.float32).T)
    w3T_h = np.ascontiguousarray(np.asarray(w3, np.float32).T)
    b1_h = np.ascontiguousarray(np.asarray(b1, np.float32).reshape(512, 1))
    b2_h = np.ascontiguousarray(np.asarray(b2, np.float32).reshape(C, 1))
    b3_h = np.ascontiguousarray(np.asarray(b3, np.float32).reshape(3 * K, 1))

    in_maps = []
    for c in range(NCORES):
        rows = slice(c * TOK_PER_CORE, (c + 1) * TOK_PER_CORE)
        in_maps.append({
            "xT": np.ascontiguousarray(x[rows].T),
            "cbT": cbT_h,
            "cb": cbk,
            "cnorm": cnorm_h,
            "w1T": w1T_h, "w2T": w2T_h, "w3T": w3T_h,
            "b1": b1_h, "b2": b2_h, "b3": b3_h,
            "gt": np.ascontiguousarray(gt_full[rows]),
        })

    trace = os.environ.get("KERNEL_TRACE", "0") == "1"
    if trace:
        tmpdir = "/root/problem/_trace"
        os.makedirs(tmpdir, exist_ok=True)
        try:
            res = run_bass_kernel_spmd(nc, in_maps, list(range(NCORES)),
                                       trace=True, tmpdir=tmpdir)
        except Exception as e:
            print(f"trace run failed ({e}); retrying without trace")
            res = run_bass_kernel_spmd(nc, in_maps, list(range(NCORES)))
    else:
        res = run_bass_kernel_spmd(nc, in_maps, list(range(NCORES)))
    global LAST_EXEC_TIME_NS
    LAST_EXEC_TIME_NS = res.exec_time_ns

    total = np.float64(0.0)
    for c in range(NCORES):
        total += res.results[c]["out"].astype(np.float64).sum()
    loss = total / (B * G * K)
    return np.float32(loss)


LAST_EXEC_TIME_NS = None


# revision 9
# speedup vs baseline: 1.2966x; 1.2966x over previous
"""Discrete VAE (VQ codebook) kernel for 8 Trainium2 NeuronCores.

Data-parallel over batch: 1024 tokens/core, 8 token-tiles of 128.

Per token-tile:
  scores[t,n] = sum_c x[c,t]*cb[c,n] + (-0.5*||c_n||^2)   (bf16 matmuls,
      cnorm added by a K=2 ones-matmul against bf16 hi+lo rows)
  PSUM -> SBUF evacuation on the Scalar engine, cast to bf16
  argmax via vector.max + max_index (bf16 scans)
  q = codebook[ids] via indirect DMA gather (fp32)
Per super-tile (4 token-tiles, N=512):
  feature-major MLP in bf16: h1T = relu(w1@qT+b1) ... recT = w3@h2+b3
Chamfer per token-tile: subtract/csum on GpSimd, square on Scalar,
  min-reductions on Vector; per-token mins accumulated, summed on host.
"""

import sys

if "/opt/trn_rl_repo" not in sys.path:
    sys.path.insert(0, "/opt/trn_rl_repo")

import os
import numpy as np
import ml_dtypes

from concourse import bacc, bass, mybir
from concourse.bass import IndirectOffsetOnAxis
from concourse.masks import make_identity
from concourse.tile import TileContext
from concourse.bass_utils import run_bass_kernel_spmd

B, G, K, C, NT = 128, 64, 32, 256, 8192
NCORES = 8
TOK_PER_CORE = B * G // NCORES  # 1024
NTILES = TOK_PER_CORE // 128  # 8
NCHUNK = NT // 1024  # 8 psum chunks of 1024 (2 banks each)

F32 = mybir.dt.float32
BF16 = mybir.dt.bfloat16
U32 = mybir.dt.uint32
AF = mybir.ActivationFunctionType
ALU = mybir.AluOpType

_CACHE = {}


def _build():
    if "nc" in _CACHE:
        return _CACHE["nc"]

    nc = bacc.Bacc("TRN2", target_bir_lowering=False, debug=False,
                   num_devices=NCORES)

    xT = nc.dram_tensor("xT", [C, TOK_PER_CORE], BF16, kind="ExternalInput")
    cbT = nc.dram_tensor("cbT", [C, NT], BF16, kind="ExternalInput")
    cbias = nc.dram_tensor("cbias", [2, NT], BF16, kind="ExternalInput")
    cb = nc.dram_tensor("cb", [NT, C], F32, kind="ExternalInput")
    w1T = nc.dram_tensor("w1T", [C, 512], BF16, kind="ExternalInput")
    w2T = nc.dram_tensor("w2T", [512, C], BF16, kind="ExternalInput")
    w3T = nc.dram_tensor("w3T", [C, 3 * K], BF16, kind="ExternalInput")
    b1 = nc.dram_tensor("b1", [512, 1], F32, kind="ExternalInput")
    b2 = nc.dram_tensor("b2", [C, 1], F32, kind="ExternalInput")
    b3 = nc.dram_tensor("b3", [3 * K, 1], F32, kind="ExternalInput")
    gt = nc.dram_tensor("gt", [TOK_PER_CORE, 3 * K], F32, kind="ExternalInput")
    out = nc.dram_tensor("out", [128, K], F32, kind="ExternalOutput")

    with TileContext(nc) as tc:
        with (
            tc.tile_pool(name="const", bufs=1) as cpool,
            tc.tile_pool(name="scores", bufs=2) as spool,
            tc.tile_pool(name="work", bufs=3) as wpool,
            tc.tile_pool(name="mlp", bufs=8) as mpool,
            tc.tile_pool(name="cham", bufs=3) as chpool,
            tc.tile_pool(name="ps_score", bufs=2, space="PSUM") as ps_s,
            tc.tile_pool(name="ps_mlp", bufs=2, space="PSUM") as ps_m,
            tc.tile_pool(name="ps_tr", bufs=2, space="PSUM") as ps_t,
        ):
            # ---- resident constants ----
            ident = cpool.tile([128, 128], F32, tag="ident")
            make_identity(nc, ident[:])

            cbT_sb = []
            for kk in range(2):
                t = cpool.tile([128, NT], BF16, tag=f"cbT{kk}")
                nc.gpsimd.dma_start(out=t[:], in_=cbT[kk * 128:(kk + 1) * 128, :])
                cbT_sb.append(t)
            cbias_sb = cpool.tile([2, NT], BF16, tag="cbias")
            nc.gpsimd.dma_start(out=cbias_sb[:], in_=cbias[:, :])
            ones2 = cpool.tile([2, 128], BF16, tag="ones2")
            nc.vector.memset(ones2[:], 1.0)

            w1_sb = []
            for kk in range(2):
                t = cpool.tile([128, 512], BF16, tag=f"w1_{kk}")
                nc.gpsimd.dma_start(out=t[:], in_=w1T[kk * 128:(kk + 1) * 128, :])
                w1_sb.append(t)
            w2_sb = []
            for kk in range(4):
                t = cpool.tile([128, C], BF16, tag=f"w2_{kk}")
                nc.gpsimd.dma_start(out=t[:], in_=w2T[kk * 128:(kk + 1) * 128, :])
                w2_sb.append(t)
            w3_sb = []
            for kk in range(2):
                t = cpool.tile([128, 3 * K], BF16, tag=f"w3_{kk}")
                nc.gpsimd.dma_start(out=t[:], in_=w3T[kk * 128:(kk + 1) * 128, :])
                w3_sb.append(t)
            b1_sb = []
            for m in range(4):
                t = cpool.tile([128, 1], F32, tag=f"b1_{m}")
                nc.gpsimd.dma_start(out=t[:], in_=b1[m * 128:(m + 1) * 128, :])
                b1_sb.append(t)
            b2_sb = []
            for m in range(2):
                t = cpool.tile([128, 1], F32, tag=f"b2_{m}")
                nc.gpsimd.dma_start(out=t[:], in_=b2[m * 128:(m + 1) * 128, :])
                b2_sb.append(t)
            b3_sb = cpool.tile([3 * K, 1], F32, tag="b3")
            nc.gpsimd.dma_start(out=b3_sb[:], in_=b3[:, :])

            acc = cpool.tile([128, K], F32, tag="acc")
            nc.vector.memset(acc[:], 0.0)

            for sup in range(NTILES // 4):  # super-tiles of 512 tokens
                qT_big = [mpool.tile([128, 512], BF16, tag=f"qTb{kk}",
                                     name=f"qT_big{kk}")
                          for kk in range(2)]

                for st in range(4):
                    t = sup * 4 + st
                    ts = slice(t * 128, (t + 1) * 128)

                    xt = []
                    for kk in range(2):
                        x = wpool.tile([128, 128], BF16, tag="xt")
                        nc.gpsimd.dma_start(
                            out=x[:], in_=xT[kk * 128:(kk + 1) * 128, ts])
                        xt.append(x)

                    scores = spool.tile([128, NT], BF16, tag="scores")
                    for ch in range(NCHUNK):  # 8 chunks of 1024 = 2 matmuls of 512
                        cs = slice(ch * 1024, (ch + 1) * 1024)
                        ps = ps_s.tile([128, 1024], F32, tag="ps_score")
                        for half in range(2):
                            hs = slice(ch * 1024 + half * 512,
                                       ch * 1024 + (half + 1) * 512)
                            po = ps[:, half * 512:(half + 1) * 512]
                            nc.tensor.matmul(po, lhsT=xt[0][:],
                                             rhs=cbT_sb[0][:, hs],
                                             start=True, stop=False)
                            nc.tensor.matmul(po, lhsT=xt[1][:],
                                             rhs=cbT_sb[1][:, hs],
                                             start=False, stop=False)
                            nc.tensor.matmul(po, lhsT=ones2[:],
                                             rhs=cbias_sb[:, hs],
                                             start=False, stop=True)
                        nc.scalar.activation(out=scores[:, cs], in_=ps[:],
                                             func=AF.Copy)

                    max8 = wpool.tile([128, 8], BF16, tag="max8")
                    nc.vector.max(out=max8[:], in_=scores[:])
                    idx8 = wpool.tile([128, 8], U32, tag="idx8")
                    nc.vector.max_index(out=idx8[:], in_max=max8[:],
                                        in_values=scores[:])

                    q = wpool.tile([128, C], F32, tag="q")
                    nc.gpsimd.indirect_dma_start(
                        out=q[:], out_offset=None, in_=cb[:, :],
                        in_offset=IndirectOffsetOnAxis(ap=idx8[:, 0:1], axis=0),
                    )

                    for kk in range(2):
                        pt = ps_t.tile([128, 128], F32, tag="ps_tr")
                        nc.tensor.transpose(
                            out=pt[:], in_=q[:, kk * 128:(kk + 1) * 128],
                            identity=ident[:])
                        nc.scalar.activation(
                            out=qT_big[kk][:, st * 128:(st + 1) * 128],
                            in_=pt[:], func=AF.Copy)

                # ---- MLP over 512 tokens, feature-major, bf16 ----
                h1 = []
                for m in range(4):
                    ph = ps_m.tile([128, 512], F32, tag="ps_mlp")
                    for kk in range(2):
                        nc.tensor.matmul(ph[:],
                                         lhsT=w1_sb[kk][:, m * 128:(m + 1) * 128],
                                         rhs=qT_big[kk][:],
                                         start=(kk == 0), stop=(kk == 1))
                    ht = mpool.tile([128, 512], BF16, tag="h1")
                    nc.scalar.activation(out=ht[:], in_=ph[:], func=AF.Relu,
                                         bias=b1_sb[m][:])
                    h1.append(ht)

                h2 = []
                for m in range(2):
                    ph = ps_m.tile([128, 512], F32, tag="ps_mlp")
                    for kk in range(4):
                        nc.tensor.matmul(ph[:],
                                         lhsT=w2_sb[kk][:, m * 128:(m + 1) * 128],
                                         rhs=h1[kk][:],
                                         start=(kk == 0), stop=(kk == 3))
                    ht = mpool.tile([128, 512], BF16, tag="h2")
                    nc.scalar.activation(out=ht[:], in_=ph[:], func=AF.Relu,
                                         bias=b2_sb[m][:])
                    h2.append(ht)

                pr = ps_m.tile([96, 512], F32, tag="ps_mlp")
                for kk in range(2):
                    nc.tensor.matmul(pr[:], lhsT=w3_sb[kk][:], rhs=h2[kk][:],
                                     start=(kk == 0), stop=(kk == 1))
                recT = mpool.tile([96, 512], F32, tag="recT")
                nc.scalar.activation(out=recT[:], in_=pr[:], func=AF.Identity,
                                     bias=b3_sb[:])

                # ---- chamfer per token-tile ----
                for st in range(4):
                    t = sup * 4 + st
                    ts = slice(t * 128, (t + 1) * 128)

                    prt = ps_t.tile([128, 128], F32, tag="ps_tr")
                    nc.tensor.transpose(
                        out=prt[:, 0:96],
                        in_=recT[:, st * 128:(st + 1) * 128],
                        identity=ident[0:96, 0:96])
                    rec = wpool.tile([128, 96], F32, tag="rec")
                    nc.vector.tensor_copy(out=rec[:], in_=prt[:, 0:96])

                    gtt = wpool.tile([128, 96], F32, tag="gt")
                    nc.gpsimd.dma_start(out=gtt[:], in_=gt[ts, :])

                    dif = chpool.tile([128, K * K * 3], F32, tag="dif")
                    rec_b = (rec[:].rearrange("p (i c) -> p i c", c=3)
                             .unsqueeze(2).broadcast_to([128, K, K, 3]))
                    gt_b = (gtt[:].rearrange("p (j c) -> p j c", c=3)
                            .unsqueeze(1).broadcast_to([128, K, K, 3]))
                    dif4 = dif[:].rearrange("p (i j c) -> p i j c", j=K, c=3)
                    nc.gpsimd.tensor_tensor(out=dif4, in0=rec_b, in1=gt_b,
                                            op=ALU.subtract)
                    nc.scalar.activation(out=dif[:], in_=dif[:], func=AF.Square)

                    # dd[i,j] = sum_c dif^2 : two strided adds on gpsimd
                    dd = chpool.tile([128, K * K], F32, tag="dd")
                    difc = dif[:].rearrange("p (ij c) -> p ij c", c=3)
                    nc.gpsimd.tensor_tensor(out=dd[:], in0=difc[:, :, 0],
                                            in1=difc[:, :, 1], op=ALU.add)
                    nc.gpsimd.tensor_tensor(out=dd[:], in0=dd[:],
                                            in1=difc[:, :, 2], op=ALU.add)

                    dd3 = dd[:].rearrange("p (i j) -> p i j", j=K)
                    minj = chpool.tile([128, K], F32, tag="minj")
                    nc.vector.tensor_reduce(out=minj[:], in_=dd3,
                                            axis=mybir.AxisListType.X,
                                            op=ALU.min)
                    mini = chpool.tile([128, K], F32, tag="mini")
                    nc.vector.tensor_reduce(out=mini[:], in_=dd3.transpose([0, 2, 1]),
                                            axis=mybir.AxisListType.X,
                                            op=ALU.min)
                    nc.vector.tensor_tensor(out=acc[:], in0=acc[:], in1=minj[:],
                                            op=ALU.add)
                    nc.vector.tensor_tensor(out=acc[:], in0=acc[:], in1=mini[:],
                                            op=ALU.add)

            nc.gpsimd.dma_start(out=out[:, :], in_=acc[:])

    nc.compile()
    _CACHE["nc"] = nc
    return nc


def kernel(patch_features, neighborhood, codebook, w1, b1, w2, b2, w3, b3):
    nc = _build()
    bf = ml_dtypes.bfloat16

    x = np.ascontiguousarray(
        np.asarray(patch_features, np.float32).reshape(B * G, C))
    gt_full = np.ascontiguousarray(
        np.asarray(neighborhood, np.float32).reshape(B * G, 3 * K))
    cbk = np.ascontiguousarray(np.asarray(codebook, np.float32))
    cbT_h = np.ascontiguousarray(cbk.T.astype(bf))
    cn32 = (-0.5 * (cbk.astype(np.float64) ** 2).sum(1)).astype(np.float32)
    ch = cn32.astype(bf)
    cl = (cn32 - ch.astype(np.float32)).astype(bf)
    cbias_h = np.ascontiguousarray(np.stack([ch, cl]))
    w1T_h = np.ascontiguousarray(np.asarray(w1, np.float32).T.astype(bf))
    w2T_h = np.ascontiguousarray(np.asarray(w2, np.float32).T.astype(bf))
    w3T_h = np.ascontiguousarray(np.asarray(w3, np.float32).T.astype(bf))
    b1_h = np.ascontiguousarray(np.asarray(b1, np.float32).reshape(512, 1))
    b2_h = np.ascontiguousarray(np.asarray(b2, np.float32).reshape(C, 1))
    b3_h = np.ascontiguousarray(np.asarray(b3, np.float32).reshape(3 * K, 1))

    in_maps = []
    for c in range(NCORES):
        rows = slice(c * TOK_PER_CORE, (c + 1) * TOK_PER_CORE)
        in_maps.append({
            "xT": np.ascontiguousarray(x[rows].T.astype(bf)),
            "cbT": cbT_h,
            "cbias": cbias_h,
            "cb": cbk,
            "w1T": w1T_h, "w2T": w2T_h, "w3T": w3T_h,
            "b1": b1_h, "b2": b2_h, "b3": b3_h,
            "gt": np.ascontiguousarray(gt_full[rows]),
        })

    trace = os.environ.get("KERNEL_TRACE", "0") == "1"
    if trace:
        tmpdir = "/root/problem/_trace"
        os.makedirs(tmpdir, exist_ok=True)
        try:
            res = run_bass_kernel_spmd(nc, in_maps, list(range(NCORES)),
                                       trace=True, tmpdir=tmpdir)
        except Exception as e:
            print(f"trace run failed ({e}); retrying without trace")
            res = run_bass_kernel_spmd(nc, in_maps, list(range(NCORES)))
    else:
        res = run_bass_kernel_spmd(nc, in_maps, list(range(NCORES)))
    global LAST_EXEC_TIME_NS
    LAST_EXEC_TIME_NS = res.exec_time_ns

    total = np.float64(0.0)
    for c in range(NCORES):
        total += res.results[c]["out"].astype(np.float64).sum()
    loss = total / (B * G * K)
    return np.float32(loss)


LAST_EXEC_TIME_NS = None
